# revision 25
# baseline (speedup 1.0000x reference)
"""Trainium2 Bass kernel for nn_ATTEfficient (ragged segment attention pooling).

reference:
    H = tanh(features @ Ww.T + bw)          # [TOTAL, D]
    s = H @ v                                # [TOTAL]
    att = segment_softmax(s, segment_ids)    # [TOTAL]
    pooled = segment_sum(features * att)     # [N_SEG, D]
    h = relu(pooled @ W1.T + b1)             # [N_SEG, D_HEAD]
    out = h @ W2.T + b2                      # [N_SEG, 1]

Sharding: tokens split into 8 contiguous ranges of exactly TOTAL/8, cutting
segments at range boundaries (softmax uses no max-subtraction so per-part
exp sums combine exactly); weights replicated. Each core computes pooled/z
partials and the full head for its local segments; the host sums the <=7
boundary-segment partials and redoes their tiny head rows in numpy.

Device pipeline per core, token-partition layout, one 128-token chunk at a
time:
    - H chunk [128 tok, 1280] = X.T-pair stationary (fp8 DoubleRow, W
      pre-scaled x64 on host) x Ww.T moving, accumulated in 3 PSUM column
      groups; ACT tanh (scale 1/64) -> ht bf16
    - s row via ONE fused DVE scalar_tensor_tensor (ht * v-broadcast with
      accum_out = per-partition sum) -> per-token s, ACT exp
    - A = onehot_mask * e (DVE); pooling matmuls (bf16) accumulate
      pooled[seg,:] across chunks into a 3-bank PSUM group; z rides as two
      ones-columns appended to the xn tiles (no separate z matmul)
  tail: z clamp/recip, pooled f32->bf16 cast split across DVE+ACT, 10 bf16
  PE transposes overlapped with the 10 bf16 head matmuls, relu, fused DVE
  dot with W2, 1/z scale, out DMA'd straight from partition layout [S,1].

Startup: zeros built by on-device memset (PE warm-up zero-matmuls depend
on no DMA); initial weight/data DMAs are few/large and split across BOTH
HW DMA queues (Sync + Scalar), chunk-0-critical pieces first.

fp8 is e4m3 on X and 64*Ww only; pooling/head stay bf16.
"""

import os
import numpy as np
import ml_dtypes

import concourse.bass as bass
import concourse.tile as tile
from concourse import bacc, mybir
from concourse.bass_utils import run_bass_kernel_spmd
from concourse.masks import make_identity

F32 = mybir.dt.float32
BF16 = mybir.dt.bfloat16
FP8 = mybir.dt.float8e4
AF = mybir.ActivationFunctionType
ALU = mybir.AluOpType
NPF8 = ml_dtypes.float8_e4m3
NPBF = ml_dtypes.bfloat16

N_CORES = 8
N_SEG = 128
D = 1280
KB = D // 128   # 10 feature blocks
DH = 512
SP = 32         # padded segment-partition count
WSCALE = 64.0   # fp8 weight pre-scale (undone by ACT tanh input scale)

USE_FP8 = bool(int(os.environ.get("KERNEL_FP8", "1")))  # False -> bf16 H
USE_STT = bool(int(os.environ.get("KERNEL_STT", "1")))  # fused mul+reduce
NEW_TAIL = bool(int(os.environ.get("KERNEL_NEWTAIL", "1")))
SPLIT = bool(int(os.environ.get("KERNEL_SPLIT", "1")))  # exact token split
WARM_REPS = int(os.environ.get("KERNEL_WARM", "5"))
WARMKEEP = bool(int(os.environ.get("KERNEL_WARMKEEP", "1")))
TAIL_PE = bool(int(os.environ.get("KERNEL_TAIL_PE", "1")))  # old-tail mode

LAST_RESULTS = None
_PROGRAM_CACHE = {}


def _partition_segments(lengths: np.ndarray) -> list[int]:
    """Split N_SEG contiguous segments into N_CORES contiguous groups
    minimizing the max token count (binary search + greedy packing)."""
    lengths = lengths.astype(np.int64)
    total = int(lengths.sum())

    def cuts_for(cap):
        cuts = [0]
        cur = 0
        for i, L in enumerate(lengths):
            if cur + L > cap and cur > 0:
                cuts.append(i)
                cur = 0
                if len(cuts) > N_CORES:
                    return None
            cur += int(L)
        while len(cuts) < N_CORES:
            cuts.append(N_SEG)
        cuts.append(N_SEG)
        return cuts

    lo, hi = max(int(lengths.max()), (total + N_CORES - 1) // N_CORES), total
    while lo < hi:
        mid = (lo + hi) // 2
        if cuts_for(mid) is not None:
            hi = mid
        else:
            lo = mid + 1
    return cuts_for(lo)


def _emit(tc: tile.TileContext, t: dict, T_pad: int, S: int,
          b1_zero: bool, bw_zero: bool):
    nc = tc.nc
    NB = T_pad // 128
    GRP = ((0, 512), (512, 512), (1024, 256))  # H psum column groups
    # pooling groups over xn cols; last is 258 = blocks 8,9 + 2 ones-cols
    # feeding the z accumulator at pooled block 10 cols 0:2
    PGRP = ((0, 512), (512, 512), (1024, 258))
    DP = D + 2
    mmdt = FP8 if USE_FP8 else BF16

    with tc.tile_pool(name="const", bufs=1) as cp:
        # zeros built on-device: the PE warm-up matmuls depend on no DMA
        # and start right after the framework preamble
        zo_sb = cp.tile([128, 512], BF16)
        nc.gpsimd.memset(zo_sb[:], 0.0)
        wwt_sb = cp.tile([128, KB, D], mmdt)
        # chunk-major X.T: [part, chunk, kb, col] — every DMA is a flat
        # contiguous copy (2KB+ packets), and the H stationary slice for
        # chunk c pair kp is xt_sb[:, c, 2kp:2kp+2, :]
        xt_sb = cp.tile([128, NB, KB, 128], mmdt)
        vrep_sb = cp.tile([128, D], BF16)
        m1h_sb = cp.tile([128, NB, S], BF16)
        e_sb = cp.tile([128, NB], F32)
        w1t_sb = cp.tile([128, KB, DH], BF16)
        w2b_sb = cp.tile([SP, DH], BF16)
        bwrep_sb = cp.tile([128, D], F32)
        b1rep_sb = cp.tile([SP, DH], F32)

        with tc.tile_pool(name="accps", bufs=1, space="PSUM") as accp:
            # pooled accumulator: blocks 0:10 pooled (d), block 10 cols
            # 0:2 hold z
            pooled_ps = accp.tile([SP, 12, 128], F32)

            # zero-matmuls: order-safe accumulator init + HAM warm while the
            # startup weight/data DMAs stream in
            for _rep in range(WARM_REPS):
                for a0 in (0, 4, 8):
                    nc.tensor.matmul(pooled_ps[:, a0:a0 + 4, :],
                                     zo_sb[:, 0:SP], zo_sb[:, 0:512],
                                     start=True, stop=False,
                                     skip_group_check=True)

            with tc.tile_pool(name="hps2", bufs=2, space="PSUM") as hps2, \
                 tc.tile_pool(name="hps1", bufs=1, space="PSUM") as hps1, \
                 tc.tile_pool(name="xnp", bufs=7) as xnp, \
                 tc.tile_pool(name="htp", bufs=2) as htp, \
                 tc.tile_pool(name="prodp", bufs=2) as prodp, \
                 tc.tile_pool(name="sp", bufs=4) as sp_pool, \
                 tc.tile_pool(name="ap", bufs=3) as ap_pool:

                xn_tiles = {}
                pend = {}
                spend = {}

                def emit_expA(c):
                    s_sb = spend.pop(c)
                    nc.scalar.activation(e_sb[:, c:c + 1], s_sb[:], AF.Exp)
                    A_sb = ap_pool.tile([128, S], BF16)
                    nc.vector.tensor_scalar_mul(
                        A_sb[:], m1h_sb[:, c, 0:S], e_sb[:, c:c + 1])
                    pend[c] = (A_sb, xn_tiles.pop(c))

                def emit_pool(c):
                    A_sb, xn_sb = pend.pop(c)
                    last = c == NB - 1
                    # flat [SP, 1536] view so the last group (width 258)
                    # can span blocks 8,9 plus the z columns at block 10
                    pooled_flat = pooled_ps[:, :, :].rearrange(
                        "s b c -> s (b c)")
                    for c0, cw in PGRP:
                        nc.tensor.matmul(
                            pooled_flat[0:S, c0:c0 + cw],
                            A_sb[:, 0:S], xn_sb[:, c0:c0 + cw], start=False,
                            stop=(last and c0 + cw == DP),
                            skip_group_check=True)

                def _pool_out(pooled_ps, S, c0, cw):
                    # view [S, cw] at flat col offset c0 of the 12*128 block
                    return pooled_ps.rearrange("s b c -> s (b c)")[
                        0:S, c0:c0 + cw]

                for c in range(NB):
                    if c == 0:
                        # startup DMAs: few/large, split across the two HW
                        # DMA queues, chunk-0-critical pieces first.
                        # sync: xt cols 0:128, wwt pairs 0/2/4, xt 128:512
                        # scalar: v-bcast, wwt pairs 1/3, m1h, bwr
                        CW = KB * 128  # flat xt cols per chunk

                        def _wwt_pair(eng, kp):
                            eng.dma_start(
                                out=wwt_sb[:, 2 * kp:2 * kp + 2, :],
                                in_=t["wwt"][2 * kp * 128:
                                             (2 * kp + 2) * 128, :]
                                    .rearrange("(kb p) n -> p kb n", p=128))

                        def _xt_chunks(eng, ca, cb):
                            eng.dma_start(
                                out=xt_sb[:, ca:cb, :, :],
                                in_=t["xt"][:, ca * CW:cb * CW])
                        _xt_chunks(nc.sync, 0, 1)
                        _wwt_pair(nc.scalar, 0)
                        _wwt_pair(nc.sync, 1)
                        _wwt_pair(nc.scalar, 2)
                        if NB > 1:
                            _xt_chunks(nc.sync, 1, 2)
                        _wwt_pair(nc.scalar, 3)
                        _wwt_pair(nc.sync, 4)
                        nc.scalar.dma_start(out=vrep_sb[0:1, :],
                                            in_=t["vr"][0:1, :])
                        nc.gpsimd.partition_broadcast(vrep_sb[:],
                                                      vrep_sb[0:1, :])
                        if NB > 2:
                            _xt_chunks(nc.sync, 2, min(4, NB))
                        nc.scalar.dma_start(
                            out=m1h_sb[:],
                            in_=t["m1h"].rearrange("p (nb s) -> p nb s", s=S))
                        if bw_zero:
                            # tensor referenced but contents unused: 1 row
                            nc.scalar.dma_start(out=bwrep_sb[0:1, :],
                                                in_=t["bwr"][0:1, :])
                        else:
                            nc.scalar.dma_start(out=bwrep_sb[:],
                                                in_=t["bwr"][:])
                    if c == NB // 2:
                        # tail-only weights: load mid-loop, clear of the
                        # startup DMA burst
                        nc.sync.dma_start(
                            out=w1t_sb[:],
                            in_=t["w1t"].rearrange("(kb p) m -> p kb m", p=128))
                        nc.sync.dma_start(out=w2b_sb[:], in_=t["w2r"][:])
                        if b1_zero:
                            nc.sync.dma_start(out=b1rep_sb[0:1, :],
                                              in_=t["b1r"][0:1, :])
                        else:
                            nc.sync.dma_start(out=b1rep_sb[:],
                                              in_=t["b1r"][:])
                    # xt chunk prefetch: one flat contiguous DMA per 4
                    # chunks (chunks 0..3 primed at c==0), ~4-chunk lead
                    if c % 4 == 0 and c + 4 < NB:
                        _xt_chunks(nc.sync, c + 4, min(c + 8, NB))
                    # xn prefetch ahead of pooling use; last two columns are
                    # ones (memset) feeding the z accumulation
                    for pc in ([0, 1, 2] if c == 0 else
                               ([c + 2] if c + 2 < NB else [])):
                        xn_sb = xnp.tile([128, DP], BF16)
                        nc.gpsimd.memset(xn_sb[:, D:DP], 1.0)
                        nc.scalar.dma_start(
                            out=xn_sb[:, 0:D],
                            in_=t["xn"][pc * 128:(pc + 1) * 128, :])
                        xn_tiles[pc] = xn_sb

                    # deferred stages for older chunks go FIRST in each
                    # engine's FIFO so no op ever waits at a queue head on a
                    # fresh cross-engine dependency:
                    #   ACT: exp(c-2) before tanh(c); DVE: A(c-2) before
                    #   mul(c); PE: pooling(c-3) before H(c)
                    if c >= 2:
                        emit_expA(c - 2)
                    if c >= 3:
                        emit_pool(c - 3)

                    # H matmuls: fp8 DoubleRow over kb pairs (stationary =
                    # X.T pair, reused across the 3 column groups)
                    h_ps = [hps2.tile([128, 512], F32, name="hg0"),
                            hps2.tile([128, 512], F32, name="hg1"),
                            hps1.tile([128, 512], F32, name="hg2")]
                    if USE_FP8:
                        for kp in range(KB // 2):
                            for g, (c0, cw) in enumerate(GRP):
                                nc.tensor.matmul(
                                    h_ps[g][:, 0:cw],
                                    xt_sb[:, c, 2 * kp:2 * kp + 2, :],
                                    wwt_sb[:, 2 * kp:2 * kp + 2, c0:c0 + cw],
                                    start=(kp == 0), stop=(kp == KB // 2 - 1),
                                    perf_mode=mybir.MatmulPerfMode.DoubleRow)
                    else:
                        for kb in range(KB):
                            for g, (c0, cw) in enumerate(GRP):
                                nc.tensor.matmul(
                                    h_ps[g][:, 0:cw],
                                    xt_sb[:, c, kb, :],
                                    wwt_sb[:, kb, c0:c0 + cw],
                                    start=(kb == 0), stop=(kb == KB - 1))

                    if not bw_zero:
                        for g, (c0, cw) in enumerate(GRP):
                            nc.vector.tensor_add(h_ps[g][:, 0:cw],
                                                 h_ps[g][:, 0:cw],
                                                 bwrep_sb[:, c0:c0 + cw])
                    ht_sb = htp.tile([128, D], BF16)
                    for g in (2, 0, 1):
                        c0, cw = GRP[g]
                        nc.scalar.activation(ht_sb[:, c0:c0 + cw],
                                             h_ps[g][:, 0:cw], AF.Tanh,
                                             scale=1.0 / WSCALE)
                    # s = sum over features of ht * v
                    prod_sb = prodp.tile([128, D], BF16)
                    s_sb = sp_pool.tile([128, 1], F32)
                    if USE_STT and c == NB - 1:
                        # last chunk: per-group fused ops right behind each
                        # tanh shorten the end-of-loop drain
                        s3_sb = sp_pool.tile([128, 4], F32, name="s3")
                        for g in (2, 0, 1):
                            c0, cw = GRP[g]
                            nc.vector.scalar_tensor_tensor(
                                out=prod_sb[:, c0:c0 + cw],
                                in0=ht_sb[:, c0:c0 + cw], scalar=0.0,
                                in1=vrep_sb[:, c0:c0 + cw],
                                op0=ALU.bypass, op1=ALU.mult,
                                accum_out=s3_sb[:, g:g + 1])
                        nc.vector.tensor_reduce(s_sb[:], s3_sb[:, 0:3],
                                                axis=mybir.AxisListType.X,
                                                op=ALU.add)
                    elif USE_STT:
                        # single fused DVE op: prod = ht * vrep,
                        # accum_out = row-sum(prod)
                        nc.vector.scalar_tensor_tensor(
                            out=prod_sb[:], in0=ht_sb[:], scalar=0.0,
                            in1=vrep_sb[:], op0=ALU.bypass, op1=ALU.mult,
                            accum_out=s_sb[:])
                    else:
                        nc.vector.tensor_mul(prod_sb[:], ht_sb[:],
                                             vrep_sb[:])
                        nc.vector.tensor_reduce(s_sb[:], prod_sb[:],
                                                axis=mybir.AxisListType.X,
                                                op=ALU.add)
                    spend[c] = s_sb

                def warm_mm(pool, name):
                    # dummy zero-matmul into a released h slot: fills PE
                    # idle during the end-of-loop drain so HAM stays at
                    # K=8/8 for the tail matmuls
                    w_ps = pool.tile([128, 512], F32, name=name)
                    nc.tensor.matmul(w_ps[0:SP, 0:512], zo_sb[:, 0:SP],
                                     zo_sb[:, 0:512], start=True, stop=True,
                                     skip_group_check=True)

                emit_expA(NB - 2)
                emit_expA(NB - 1)
                if WARMKEEP:
                    warm_mm(hps2, "hg0")
                    warm_mm(hps2, "hg1")
                emit_pool(NB - 3)
                if WARMKEEP:
                    warm_mm(hps2, "hg0")
                    warm_mm(hps2, "hg1")
                    warm_mm(hps1, "hg2")
                emit_pool(NB - 2)
                if WARMKEEP:
                    warm_mm(hps2, "hg0")
                    warm_mm(hps2, "hg1")
                    warm_mm(hps1, "hg2")
                emit_pool(NB - 1)

            # ---- tail / logits head ----
            zc_sb = cp.tile([SP, 1], F32)
            nc.vector.tensor_scalar_max(zc_sb[:], pooled_ps[0:SP, 10, 0:1],
                                        1e-30)
            rz_sb = cp.tile([SP, 1], F32)
            nc.vector.reciprocal(rz_sb[:], zc_sb[:])
            if SPLIT:
                nc.scalar.dma_start(out=t["zs"][:], in_=zc_sb[0:SP, 0:1])

            identS = cp.tile([SP, SP], F32)
            make_identity(nc, identS[:])
            pT_sb = cp.tile([128, KB, SP], BF16)
            if NEW_TAIL and b1_zero:
                # relu commutes with the positive per-segment 1/z scale when
                # b1 == 0: head runs on UNNORMALIZED pooled, rz applied last.
                # pooled f32->bf16 cast split across DVE+ACT, then cheap
                # bf16 PE transposes overlapped with the head matmuls.
                identS_bf = cp.tile([SP, SP], BF16)
                make_identity(nc, identS_bf[:])
                psrcb_sb = cp.tile([SP, KB, 128], BF16)
                nc.vector.tensor_copy(psrcb_sb[:, 0:5, :],
                                      pooled_ps[0:SP, 0:5, :])
                nc.scalar.copy(psrcb_sb[:, 5:KB, :],
                               pooled_ps[0:SP, 5:KB, :])
                with tc.tile_pool(name="ptps", bufs=2, space="PSUM") as ptp, \
                     tc.tile_pool(name="warmp", bufs=1,
                                  space="PSUM") as warmp:

                    def warm_tail():
                        w_ps = warmp.tile([128, 512], F32, name="wt")
                        nc.tensor.matmul(w_ps[0:SP, 0:512], zo_sb[:, 0:SP],
                                         zo_sb[:, 0:512], start=True,
                                         stop=True, skip_group_check=True)

                    if WARMKEEP:
                        warm_tail()
                        warm_tail()
                    for db in range(KB):
                        if WARMKEEP and db % 2 == 0:
                            warm_tail()
                        pT_ps = ptp.tile([128, SP], BF16)
                        nc.tensor.transpose(pT_ps[:], psrcb_sb[:, db, :],
                                            identS_bf[:])
                        # alternate copy engines so transposes stream
                        if db % 2 == 0:
                            nc.scalar.copy(pT_sb[:, db, 0:SP], pT_ps[:])
                        else:
                            nc.vector.tensor_copy(pT_sb[:, db, 0:SP],
                                                  pT_ps[:])
            elif TAIL_PE:
                psrc_sb = cp.tile([SP, KB, 128], F32)
                for a0, a1 in ((0, 4), (4, 8), (8, 10)):
                    if b1_zero:
                        nc.scalar.copy(psrc_sb[:, a0:a1, :],
                                       pooled_ps[0:SP, a0:a1, :])
                    else:
                        nc.vector.tensor_scalar_mul(
                            psrc_sb[:, a0:a1, :],
                            pooled_ps[0:SP, a0:a1, :], rz_sb[:])
                with tc.tile_pool(name="ptps", bufs=2, space="PSUM") as ptp:
                    for db in range(KB):
                        pT_ps = ptp.tile([128, SP], F32)
                        nc.tensor.transpose(pT_ps[:], psrc_sb[:, db, :],
                                            identS[:])
                        nc.scalar.copy(pT_sb[:, db, 0:SP], pT_ps[:])
            else:
                pTf_sb = cp.tile([128, KB, SP], F32)
                pn_sb = cp.tile([SP, KB, 128], F32)
                nc.vector.tensor_scalar_mul(
                    pn_sb[:, :, :], pooled_ps[0:SP, 0:KB, :], rz_sb[:])
                for t_ in range(KB):
                    for a in range(4):
                        nc.vector.transpose(
                            pTf_sb[32 * a:32 * a + 32, t_, :],
                            pn_sb[0:SP, t_, 32 * a:32 * a + 32])
                nc.scalar.copy(pT_sb[:, :, :], pTf_sb[:, :, :])

        if SPLIT:
            # boundary-segment partials for the host combine (raw bf16
            # pooled^T and clamped z); overlaps the head matmuls
            nc.scalar.dma_start(
                out=t["pt"][:],
                in_=pT_sb[:, :, :].rearrange("p kb s -> p (kb s)"))
        with tc.tile_pool(name="headps", bufs=2, space="PSUM") as headp:
            HH = DH // 2
            hnh_ps = [headp.tile([SP, HH], F32, name="hna"),
                      headp.tile([SP, HH], F32, name="hnb")]
            hn_sb = cp.tile([SP, DH], BF16)
            warmh_ps = headp.tile([SP, 512], F32, name="warmh")
            for hf in range(2):
                for db in range(KB):
                    if WARMKEEP and db % 5 == 0:
                        # transposes/copies don't count as PE-busy for HAM;
                        # keep K=8/8 through the head phase
                        nc.tensor.matmul(warmh_ps[:], zo_sb[:, 0:SP],
                                         zo_sb[:, 0:512], start=True,
                                         stop=True, skip_group_check=True)
                    nc.tensor.matmul(hnh_ps[hf][:],
                                     pT_sb[:, db, 0:SP],
                                     w1t_sb[:, db, hf * HH:(hf + 1) * HH],
                                     start=(db == 0), stop=(db == KB - 1))
                if not b1_zero:
                    nc.vector.tensor_add(hnh_ps[hf][:], hnh_ps[hf][:],
                                         b1rep_sb[:, hf * HH:(hf + 1) * HH])
                # relu of half A overlaps half B's matmuls (distinct banks)
                nc.scalar.activation(hn_sb[:, hf * HH:(hf + 1) * HH],
                                     hnh_ps[hf][:], AF.Relu)
            prodh_sb = cp.tile([SP, DH], BF16)
            oraw_sb = cp.tile([SP, 1], F32)
            if USE_STT and b1_zero:
                # fold the deferred 1/z scale into the fused dot:
                # accum = sum((hn * rz) * w2) = out
                oval_sb = cp.tile([SP, 1], F32)
                nc.vector.scalar_tensor_tensor(
                    out=prodh_sb[:], in0=hn_sb[:], scalar=rz_sb[:],
                    in1=w2b_sb[:], op0=ALU.mult, op1=ALU.mult,
                    accum_out=oval_sb[:])
            elif USE_STT:
                nc.vector.scalar_tensor_tensor(
                    out=prodh_sb[:], in0=hn_sb[:], scalar=0.0,
                    in1=w2b_sb[:], op0=ALU.bypass, op1=ALU.mult,
                    accum_out=oraw_sb[:])
                oval_sb = oraw_sb
            else:
                nc.vector.tensor_mul(prodh_sb[:], hn_sb[:], w2b_sb[:])
                nc.vector.tensor_reduce(oraw_sb[:], prodh_sb[:],
                                        axis=mybir.AxisListType.X,
                                        op=ALU.add)
                if b1_zero:
                    oval_sb = cp.tile([SP, 1], F32)
                    nc.vector.tensor_mul(oval_sb[:], oraw_sb[:], rz_sb[:])
                else:
                    oval_sb = oraw_sb
            # out straight from partition layout [S, 1]
            nc.sync.dma_start(out=t["out"][0:S, 0:1], in_=oval_sb[0:S, 0:1])


def _build_program(T_pad: int, S: int, b1_zero: bool, bw_zero: bool):
    key = (T_pad, S, b1_zero, bw_zero, USE_FP8, TAIL_PE, USE_STT, NEW_TAIL,
           WARM_REPS, SPLIT)
    if key in _PROGRAM_CACHE:
        return _PROGRAM_CACHE[key]
    NB = T_pad // 128
    mmdt = FP8 if USE_FP8 else BF16
    nc = bacc.Bacc("TRN2", target_bir_lowering=False, debug=False,
                   num_devices=N_CORES)
    t = {
        "xt": nc.dram_tensor("xt", [128, NB * KB * 128], mmdt,
                             kind="ExternalInput").ap(),
        "xn": nc.dram_tensor("xn", [T_pad, D], BF16, kind="ExternalInput").ap(),
        "wwt": nc.dram_tensor("wwt", [D, D], mmdt, kind="ExternalInput").ap(),
        "m1h": nc.dram_tensor("m1h", [128, NB * S], BF16,
                              kind="ExternalInput").ap(),
        "vr": nc.dram_tensor("vr", [128, D], BF16, kind="ExternalInput").ap(),
        "bwr": nc.dram_tensor("bwr", [128, D], F32,
                              kind="ExternalInput").ap(),
        "w1t": nc.dram_tensor("w1t", [D, DH], BF16, kind="ExternalInput").ap(),
        "b1r": nc.dram_tensor("b1r", [SP, DH], F32, kind="ExternalInput").ap(),
        "w2r": nc.dram_tensor("w2r", [SP, DH], BF16,
                              kind="ExternalInput").ap(),
        "out": nc.dram_tensor("out", [SP, 1], F32, kind="ExternalOutput").ap(),
    }
    if SPLIT:
        t["pt"] = nc.dram_tensor("pt", [128, KB * SP], BF16,
                                 kind="ExternalOutput").ap()
        t["zs"] = nc.dram_tensor("zs", [SP, 1], F32,
                                 kind="ExternalOutput").ap()
    with tile.TileContext(nc) as tc:
        _emit(tc, t, T_pad, S, b1_zero, bw_zero)
    nc.compile()
    _PROGRAM_CACHE[key] = nc
    return nc


def kernel(features, Ww, bw, v, W1, b1, W2, b2, segment_ids):
    global LAST_RESULTS
    features = np.ascontiguousarray(np.asarray(features, dtype=np.float32))
    Ww = np.asarray(Ww, dtype=np.float32)
    bw = np.asarray(bw, dtype=np.float32)
    v = np.asarray(v, dtype=np.float32)
    W1 = np.asarray(W1, dtype=np.float32)
    b1 = np.asarray(b1, dtype=np.float32)
    W2 = np.asarray(W2, dtype=np.float32)
    b2 = np.asarray(b2, dtype=np.float32)
    segment_ids = np.asarray(segment_ids)

    seg64 = segment_ids.astype(np.int64)
    total = seg64.shape[0]
    lengths = np.bincount(seg64, minlength=N_SEG)[:N_SEG]
    seg_prefix = np.concatenate([[0], np.cumsum(lengths)])

    if SPLIT and total % N_CORES == 0:
        # exact contiguous token ranges; segments may split at boundaries
        tok_cuts = [total // N_CORES * c for c in range(N_CORES + 1)]
        seg_lo = [int(seg64[tok_cuts[c]]) for c in range(N_CORES)]
        seg_hi = [int(seg64[tok_cuts[c + 1] - 1]) for c in range(N_CORES)]
        S = max(seg_hi[c] - seg_lo[c] + 1 for c in range(N_CORES))
    else:
        cuts = _partition_segments(lengths)
        tok_cuts = [int(seg_prefix[c]) for c in cuts]
        seg_lo = [cuts[c] for c in range(N_CORES)]
        seg_hi = [cuts[c + 1] - 1 for c in range(N_CORES)]
        S = max(cuts[c + 1] - cuts[c] for c in range(N_CORES))
    assert S <= SP, f"segments per core {S} exceeds {SP}"
    T_max = max(tok_cuts[c + 1] - tok_cuts[c] for c in range(N_CORES))
    T_pad = max(512, ((T_max + 127) // 128) * 128)
    NB = T_pad // 128

    b1_zero = bool(np.all(b1 == 0))
    bw_zero = bool(np.all(bw == 0))

    mmnp = NPF8 if USE_FP8 else NPBF
    wsc = WSCALE if USE_FP8 else 1.0
    wwt = np.ascontiguousarray((Ww.T * wsc)).astype(mmnp)      # [k, m]
    vr = np.tile(v.reshape(1, D), (128, 1)).astype(NPBF)
    bwr = np.tile((bw.reshape(1, D) * wsc), (128, 1)).astype(np.float32)
    w1t = np.ascontiguousarray(W1.T).astype(NPBF)              # [k, h]
    b1r = np.tile(b1.reshape(1, DH), (SP, 1)).astype(np.float32)
    w2r = np.tile(W2[0:1, :], (SP, 1)).astype(NPBF)

    in_maps = []
    for c in range(N_CORES):
        t0, t1 = tok_cuts[c], tok_cuts[c + 1]
        Tc = t1 - t0
        xn = np.zeros((T_pad, D), dtype=np.float32)
        xn[:Tc] = features[t0:t1]
        # chunk-major X.T: xt[p, ((c*KB + kb)*128 + n)] = xn[c*128+n,
        # kb*128+p]
        xt = np.ascontiguousarray(
            xn.reshape(NB, 128, KB, 128).transpose(3, 0, 2, 1)
        ).astype(mmnp).reshape(128, NB * KB * 128)
        oh = np.zeros((T_pad, S), dtype=np.float32)
        if Tc > 0:
            loc = seg64[t0:t1] - seg_lo[c]
            ok = (loc >= 0) & (loc < S)
            oh[np.arange(Tc)[ok], loc[ok]] = 1.0
        m1h = np.ascontiguousarray(
            oh.reshape(NB, 128, S).transpose(1, 0, 2).reshape(128, NB * S)
        ).astype(NPBF)
        in_maps.append({
            "xt": xt, "xn": xn.astype(NPBF), "m1h": m1h,
            "wwt": wwt, "vr": vr, "bwr": bwr,
            "w1t": w1t, "b1r": b1r, "w2r": w2r,
        })

    nc = _build_program(T_pad, S, b1_zero, bw_zero)
    trace = bool(int(os.environ.get("KERNEL_TRACE", "0")))
    res = run_bass_kernel_spmd(nc, in_maps, core_ids=list(range(N_CORES)),
                               trace=trace)
    LAST_RESULTS = res

    out = np.zeros((N_SEG, 1), dtype=np.float32)
    for c in range(N_CORES):
        n = seg_hi[c] - seg_lo[c] + 1
        out[seg_lo[c]:seg_hi[c] + 1, 0] = res.results[c]["out"][:n, 0]

    if SPLIT and total % N_CORES == 0:
        # segments split across a core boundary: sum the pooled/z partials
        # and redo those head rows on the host
        boundary = sorted({seg_lo[c] for c in range(1, N_CORES)
                           if seg_lo[c] == seg_hi[c - 1]})
        for j in boundary:
            pooled = np.zeros(D, dtype=np.float32)
            z = 0.0
            for c in range(N_CORES):
                if seg_lo[c] <= j <= seg_hi[c]:
                    loc = j - seg_lo[c]
                    pt = np.asarray(res.results[c]["pt"],
                                    dtype=np.float32)  # [128, KB*SP]
                    pooled += pt.reshape(128, KB, SP)[:, :, loc] \
                        .T.reshape(D)
                    z += float(res.results[c]["zs"][loc, 0])
            z = max(z, 1e-30)
            h = np.maximum(pooled / z @ W1.T + b1, 0.0)
            out[j, 0] = float(h @ W2[0])
    out[:, 0] += b2[0]

    # empty segments: pooled = 0 -> out = relu(b1) @ W2.T + b2 (host patch;
    # device row may be garbage from 0 * (1/0))
    empty = lengths == 0
    if empty.any():
        out[empty, 0] = float(np.maximum(b1, 0.0) @ W2[0] + b2[0])
    return out


# revision 26
# speedup vs baseline: 1.0306x; 1.0306x over previous
"""Trainium2 Bass kernel for nn_ATTEfficient (ragged segment attention pooling).

reference:
    H = tanh(features @ Ww.T + bw)          # [TOTAL, D]
    s = H @ v                                # [TOTAL]
    att = segment_softmax(s, segment_ids)    # [TOTAL]
    pooled = segment_sum(features * att)     # [N_SEG, D]
    h = relu(pooled @ W1.T + b1)             # [N_SEG, D_HEAD]
    out = h @ W2.T + b2                      # [N_SEG, 1]

Sharding: tokens split into 8 contiguous ranges of exactly TOTAL/8, cutting
segments at range boundaries (softmax uses no max-subtraction so per-part
exp sums combine exactly); weights replicated. Each core computes pooled/z
partials and the full head for its local segments; the host sums the <=7
boundary-segment partials and redoes their tiny head rows in numpy.

Device pipeline per core, token-partition layout, one 128-token chunk at a
time:
    - H chunk [128 tok, 1280] = X.T-pair stationary (fp8 DoubleRow, W
      pre-scaled x64 on host) x Ww.T moving, accumulated in 3 PSUM column
      groups; ACT tanh (scale 1/64) -> ht bf16
    - s row via ONE fused DVE scalar_tensor_tensor (ht * v-broadcast with
      accum_out = per-partition sum) -> per-token s, ACT exp
    - A = onehot_mask * e (DVE); pooling matmuls (bf16) accumulate
      pooled[seg,:] across chunks into a 3-bank PSUM group; z rides as two
      ones-columns appended to the xn tiles (no separate z matmul)
  tail: z clamp/recip, pooled f32->bf16 cast split across DVE+ACT, 10 bf16
  PE transposes overlapped with the 10 bf16 head matmuls, relu, fused DVE
  dot with W2, 1/z scale, out DMA'd straight from partition layout [S,1].

Startup: zeros built by on-device memset (PE warm-up zero-matmuls depend
on no DMA); initial weight/data DMAs are few/large and split across BOTH
HW DMA queues (Sync + Scalar), chunk-0-critical pieces first.

fp8 is e4m3 on X and 64*Ww only; pooling/head stay bf16.
"""

import os
import numpy as np
import ml_dtypes

import concourse.bass as bass
import concourse.tile as tile
from concourse import bacc, mybir
from concourse.bass_utils import run_bass_kernel_spmd
from concourse.masks import make_identity

F32 = mybir.dt.float32
BF16 = mybir.dt.bfloat16
FP8 = mybir.dt.float8e4
AF = mybir.ActivationFunctionType
ALU = mybir.AluOpType
NPF8 = ml_dtypes.float8_e4m3
NPBF = ml_dtypes.bfloat16

N_CORES = 8
N_SEG = 128
D = 1280
KB = D // 128   # 10 feature blocks
DH = 512
SP = 32         # padded segment-partition count
WSCALE = 64.0   # fp8 weight pre-scale (undone by ACT tanh input scale)

USE_FP8 = bool(int(os.environ.get("KERNEL_FP8", "1")))  # False -> bf16 H
USE_STT = bool(int(os.environ.get("KERNEL_STT", "1")))  # fused mul+reduce
NEW_TAIL = bool(int(os.environ.get("KERNEL_NEWTAIL", "1")))
SPLIT = bool(int(os.environ.get("KERNEL_SPLIT", "1")))  # exact token split
WARM_REPS = int(os.environ.get("KERNEL_WARM", "5"))
WARMKEEP = bool(int(os.environ.get("KERNEL_WARMKEEP", "1")))
TAIL_PE = bool(int(os.environ.get("KERNEL_TAIL_PE", "1")))  # old-tail mode

LAST_RESULTS = None
_PROGRAM_CACHE = {}


def _partition_segments(lengths: np.ndarray) -> list[int]:
    """Split N_SEG contiguous segments into N_CORES contiguous groups
    minimizing the max token count (binary search + greedy packing)."""
    lengths = lengths.astype(np.int64)
    total = int(lengths.sum())

    def cuts_for(cap):
        cuts = [0]
        cur = 0
        for i, L in enumerate(lengths):
            if cur + L > cap and cur > 0:
                cuts.append(i)
                cur = 0
                if len(cuts) > N_CORES:
                    return None
            cur += int(L)
        while len(cuts) < N_CORES:
            cuts.append(N_SEG)
        cuts.append(N_SEG)
        return cuts

    lo, hi = max(int(lengths.max()), (total + N_CORES - 1) // N_CORES), total
    while lo < hi:
        mid = (lo + hi) // 2
        if cuts_for(mid) is not None:
            hi = mid
        else:
            lo = mid + 1
    return cuts_for(lo)


def _emit(tc: tile.TileContext, t: dict, T_pad: int, S: int,
          b1_zero: bool, bw_zero: bool):
    nc = tc.nc
    NB = T_pad // 128
    GRP = ((0, 512), (512, 512), (1024, 256))  # H psum column groups
    # pooling groups over xn cols; last is 258 = blocks 8,9 + 2 ones-cols
    # feeding the z accumulator at pooled block 10 cols 0:2
    PGRP = ((0, 512), (512, 512), (1024, 258))
    DP = D + 2
    mmdt = FP8 if USE_FP8 else BF16

    with tc.tile_pool(name="const", bufs=1) as cp:
        # zeros built on-device: the PE warm-up matmuls depend on no DMA
        # and start right after the framework preamble
        zo_sb = cp.tile([128, 512], BF16)
        nc.gpsimd.memset(zo_sb[:], 0.0)
        wwt_sb = cp.tile([128, KB, D], mmdt)
        # chunk-major X.T: [part, chunk, kb, col] — every DMA is a flat
        # contiguous copy (2KB+ packets), and the H stationary slice for
        # chunk c pair kp is xt_sb[:, c, 2kp:2kp+2, :]
        xt_sb = cp.tile([128, NB, KB, 128], mmdt)
        vrep_sb = cp.tile([128, D], BF16)
        m1h_sb = cp.tile([128, NB, S], BF16)
        e_sb = cp.tile([128, NB], F32)
        w1t_sb = cp.tile([128, KB, DH], BF16)
        w2b_sb = cp.tile([SP, DH], BF16)
        bwrep_sb = cp.tile([128, D], F32)
        b1rep_sb = cp.tile([SP, DH], F32)

        with tc.tile_pool(name="accps", bufs=1, space="PSUM") as accp:
            # pooled accumulator: blocks 0:10 pooled (d), block 10 cols
            # 0:2 hold z
            pooled_ps = accp.tile([SP, 12, 128], F32)

            # zero-matmuls: order-safe accumulator init + HAM warm while the
            # startup weight/data DMAs stream in
            for _rep in range(WARM_REPS):
                for a0 in (0, 4, 8):
                    nc.tensor.matmul(pooled_ps[:, a0:a0 + 4, :],
                                     zo_sb[:, 0:SP], zo_sb[:, 0:512],
                                     start=True, stop=False,
                                     skip_group_check=True)

            with tc.tile_pool(name="hps2", bufs=2, space="PSUM") as hps2, \
                 tc.tile_pool(name="hps1", bufs=1, space="PSUM") as hps1, \
                 tc.tile_pool(name="xnp", bufs=7) as xnp, \
                 tc.tile_pool(name="htp", bufs=2) as htp, \
                 tc.tile_pool(name="prodp", bufs=2) as prodp, \
                 tc.tile_pool(name="sp", bufs=4) as sp_pool, \
                 tc.tile_pool(name="ap", bufs=3) as ap_pool:

                xn_tiles = {}
                pend = {}
                spend = {}

                def emit_expA(c):
                    s_sb = spend.pop(c)
                    nc.scalar.activation(e_sb[:, c:c + 1], s_sb[:], AF.Exp)
                    A_sb = ap_pool.tile([128, S], BF16)
                    nc.vector.tensor_scalar_mul(
                        A_sb[:], m1h_sb[:, c, 0:S], e_sb[:, c:c + 1])
                    pend[c] = (A_sb, xn_tiles.pop(c))

                def emit_pool(c):
                    A_sb, xn_sb = pend.pop(c)
                    last = c == NB - 1
                    # flat [SP, 1536] view so the last group (width 258)
                    # can span blocks 8,9 plus the z columns at block 10
                    pooled_flat = pooled_ps[:, :, :].rearrange(
                        "s b c -> s (b c)")
                    for c0, cw in PGRP:
                        nc.tensor.matmul(
                            pooled_flat[0:S, c0:c0 + cw],
                            A_sb[:, 0:S], xn_sb[:, c0:c0 + cw], start=False,
                            stop=(last and c0 + cw == DP),
                            skip_group_check=True)

                def _pool_out(pooled_ps, S, c0, cw):
                    # view [S, cw] at flat col offset c0 of the 12*128 block
                    return pooled_ps.rearrange("s b c -> s (b c)")[
                        0:S, c0:c0 + cw]

                for c in range(NB):
                    if c == 0:
                        # startup DMAs: few/large, split across the two HW
                        # DMA queues, chunk-0-critical pieces first.
                        # sync: xt cols 0:128, wwt pairs 0/2/4, xt 128:512
                        # scalar: v-bcast, wwt pairs 1/3, m1h, bwr
                        CW = KB * 128  # flat xt cols per chunk

                        def _wwt_pair(eng, kp):
                            eng.dma_start(
                                out=wwt_sb[:, 2 * kp:2 * kp + 2, :],
                                in_=t["wwt"][2 * kp * 128:
                                             (2 * kp + 2) * 128, :]
                                    .rearrange("(kb p) n -> p kb n", p=128))

                        def _xt_chunks(eng, ca, cb):
                            eng.dma_start(
                                out=xt_sb[:, ca:cb, :, :],
                                in_=t["xt"][:, ca * CW:cb * CW])
                        _xt_chunks(nc.sync, 0, 1)
                        _wwt_pair(nc.scalar, 0)
                        _wwt_pair(nc.sync, 1)
                        _wwt_pair(nc.scalar, 2)
                        if NB > 1:
                            _xt_chunks(nc.sync, 1, 2)
                        _wwt_pair(nc.scalar, 3)
                        _wwt_pair(nc.sync, 4)
                        nc.scalar.dma_start(out=vrep_sb[0:1, :],
                                            in_=t["vr"][0:1, :])
                        nc.gpsimd.partition_broadcast(vrep_sb[:],
                                                      vrep_sb[0:1, :])
                        if NB > 2:
                            _xt_chunks(nc.sync, 2, min(4, NB))
                        nc.scalar.dma_start(
                            out=m1h_sb[:],
                            in_=t["m1h"].rearrange("p (nb s) -> p nb s", s=S))
                        if bw_zero:
                            # tensor referenced but contents unused: 1 row
                            nc.scalar.dma_start(out=bwrep_sb[0:1, :],
                                                in_=t["bwr"][0:1, :])
                        else:
                            nc.scalar.dma_start(out=bwrep_sb[:],
                                                in_=t["bwr"][:])
                    if c == NB // 2:
                        # tail-only weights: load mid-loop, clear of the
                        # startup DMA burst
                        nc.sync.dma_start(
                            out=w1t_sb[:],
                            in_=t["w1t"].rearrange("(kb p) m -> p kb m", p=128))
                        nc.sync.dma_start(out=w2b_sb[:], in_=t["w2r"][:])
                        if b1_zero:
                            nc.sync.dma_start(out=b1rep_sb[0:1, :],
                                              in_=t["b1r"][0:1, :])
                        else:
                            nc.sync.dma_start(out=b1rep_sb[:],
                                              in_=t["b1r"][:])
                    # xt chunk prefetch: one flat contiguous DMA per 4
                    # chunks (chunks 0..3 primed at c==0), ~4-chunk lead
                    if c % 4 == 0 and c + 4 < NB:
                        _xt_chunks(nc.sync, c + 4, min(c + 8, NB))
                    # xn prefetch ahead of pooling use; last two columns are
                    # ones (memset) feeding the z accumulation
                    for pc in ([0, 1, 2] if c == 0 else
                               ([c + 2] if c + 2 < NB else [])):
                        xn_sb = xnp.tile([128, DP], BF16)
                        nc.gpsimd.memset(xn_sb[:, D:DP], 1.0)
                        nc.sync.dma_start(
                            out=xn_sb[:, 0:D],
                            in_=t["xn"][pc * 128:(pc + 1) * 128, :])
                        xn_tiles[pc] = xn_sb

                    # deferred stages for older chunks go FIRST in each
                    # engine's FIFO so no op ever waits at a queue head on a
                    # fresh cross-engine dependency:
                    #   ACT: exp(c-2) before tanh(c); DVE: A(c-2) before
                    #   mul(c); PE: pooling(c-3) before H(c)
                    if c >= 2:
                        emit_expA(c - 2)
                    if c >= 3:
                        emit_pool(c - 3)

                    # H matmuls: fp8 DoubleRow over kb pairs (stationary =
                    # X.T pair, reused across the 3 column groups)
                    h_ps = [hps2.tile([128, 512], F32, name="hg0"),
                            hps2.tile([128, 512], F32, name="hg1"),
                            hps1.tile([128, 512], F32, name="hg2")]
                    if USE_FP8:
                        for kp in range(KB // 2):
                            for g, (c0, cw) in enumerate(GRP):
                                nc.tensor.matmul(
                                    h_ps[g][:, 0:cw],
                                    xt_sb[:, c, 2 * kp:2 * kp + 2, :],
                                    wwt_sb[:, 2 * kp:2 * kp + 2, c0:c0 + cw],
                                    start=(kp == 0), stop=(kp == KB // 2 - 1),
                                    perf_mode=mybir.MatmulPerfMode.DoubleRow)
                    else:
                        for kb in range(KB):
                            for g, (c0, cw) in enumerate(GRP):
                                nc.tensor.matmul(
                                    h_ps[g][:, 0:cw],
                                    xt_sb[:, c, kb, :],
                                    wwt_sb[:, kb, c0:c0 + cw],
                                    start=(kb == 0), stop=(kb == KB - 1))

                    if not bw_zero:
                        for g, (c0, cw) in enumerate(GRP):
                            nc.vector.tensor_add(h_ps[g][:, 0:cw],
                                                 h_ps[g][:, 0:cw],
                                                 bwrep_sb[:, c0:c0 + cw])
                    ht_sb = htp.tile([128, D], BF16)
                    for g in (2, 0, 1):
                        c0, cw = GRP[g]
                        nc.scalar.activation(ht_sb[:, c0:c0 + cw],
                                             h_ps[g][:, 0:cw], AF.Tanh,
                                             scale=1.0 / WSCALE)
                    # s = sum over features of ht * v
                    prod_sb = prodp.tile([128, D], BF16)
                    s_sb = sp_pool.tile([128, 1], F32)
                    if USE_STT and c == NB - 1:
                        # last chunk: per-group fused ops right behind each
                        # tanh shorten the end-of-loop drain
                        s3_sb = sp_pool.tile([128, 4], F32, name="s3")
                        for g in (2, 0, 1):
                            c0, cw = GRP[g]
                            nc.vector.scalar_tensor_tensor(
                                out=prod_sb[:, c0:c0 + cw],
                                in0=ht_sb[:, c0:c0 + cw], scalar=0.0,
                                in1=vrep_sb[:, c0:c0 + cw],
                                op0=ALU.bypass, op1=ALU.mult,
                                accum_out=s3_sb[:, g:g + 1])
                        nc.vector.tensor_reduce(s_sb[:], s3_sb[:, 0:3],
                                                axis=mybir.AxisListType.X,
                                                op=ALU.add)
                    elif USE_STT:
                        # single fused DVE op: prod = ht * vrep,
                        # accum_out = row-sum(prod)
                        nc.vector.scalar_tensor_tensor(
                            out=prod_sb[:], in0=ht_sb[:], scalar=0.0,
                            in1=vrep_sb[:], op0=ALU.bypass, op1=ALU.mult,
                            accum_out=s_sb[:])
                    else:
                        nc.vector.tensor_mul(prod_sb[:], ht_sb[:],
                                             vrep_sb[:])
                        nc.vector.tensor_reduce(s_sb[:], prod_sb[:],
                                                axis=mybir.AxisListType.X,
                                                op=ALU.add)
                    spend[c] = s_sb

                def warm_mm(pool, name):
                    # dummy zero-matmul into a released h slot: fills PE
                    # idle during the end-of-loop drain so HAM stays at
                    # K=8/8 for the tail matmuls
                    w_ps = pool.tile([128, 512], F32, name=name)
                    nc.tensor.matmul(w_ps[0:SP, 0:512], zo_sb[:, 0:SP],
                                     zo_sb[:, 0:512], start=True, stop=True,
                                     skip_group_check=True)

                emit_expA(NB - 2)
                emit_expA(NB - 1)
                if WARMKEEP:
                    warm_mm(hps2, "hg0")
                    warm_mm(hps2, "hg1")
                emit_pool(NB - 3)
                if WARMKEEP:
                    warm_mm(hps2, "hg0")
                    warm_mm(hps2, "hg1")
                    warm_mm(hps1, "hg2")
                emit_pool(NB - 2)
                if WARMKEEP:
                    warm_mm(hps2, "hg0")
                    warm_mm(hps2, "hg1")
                    warm_mm(hps1, "hg2")
                emit_pool(NB - 1)

            # ---- tail / logits head ----
            zc_sb = cp.tile([SP, 1], F32)
            nc.vector.tensor_scalar_max(zc_sb[:], pooled_ps[0:SP, 10, 0:1],
                                        1e-30)
            rz_sb = cp.tile([SP, 1], F32)
            nc.vector.reciprocal(rz_sb[:], zc_sb[:])
            if SPLIT:
                nc.scalar.dma_start(out=t["zs"][:], in_=zc_sb[0:SP, 0:1])

            identS = cp.tile([SP, SP], F32)
            make_identity(nc, identS[:])
            pT_sb = cp.tile([128, KB, SP], BF16)
            if NEW_TAIL and b1_zero:
                # relu commutes with the positive per-segment 1/z scale when
                # b1 == 0: head runs on UNNORMALIZED pooled, rz applied last.
                # pooled f32->bf16 cast split across DVE+ACT, then cheap
                # bf16 PE transposes overlapped with the head matmuls.
                identS_bf = cp.tile([SP, SP], BF16)
                make_identity(nc, identS_bf[:])
                psrcb_sb = cp.tile([SP, KB, 128], BF16)
                nc.vector.tensor_copy(psrcb_sb[:, 0:5, :],
                                      pooled_ps[0:SP, 0:5, :])
                nc.scalar.copy(psrcb_sb[:, 5:KB, :],
                               pooled_ps[0:SP, 5:KB, :])
                with tc.tile_pool(name="ptps", bufs=2, space="PSUM") as ptp, \
                     tc.tile_pool(name="warmp", bufs=1,
                                  space="PSUM") as warmp:

                    def warm_tail():
                        w_ps = warmp.tile([128, 512], F32, name="wt")
                        nc.tensor.matmul(w_ps[0:SP, 0:512], zo_sb[:, 0:SP],
                                         zo_sb[:, 0:512], start=True,
                                         stop=True, skip_group_check=True)

                    if WARMKEEP:
                        warm_tail()
                        warm_tail()
                    for db in range(KB):
                        if WARMKEEP and db % 2 == 0:
                            warm_tail()
                        pT_ps = ptp.tile([128, SP], BF16)
                        nc.tensor.transpose(pT_ps[:], psrcb_sb[:, db, :],
                                            identS_bf[:])
                        # alternate copy engines so transposes stream
                        if db % 2 == 0:
                            nc.scalar.copy(pT_sb[:, db, 0:SP], pT_ps[:])
                        else:
                            nc.vector.tensor_copy(pT_sb[:, db, 0:SP],
                                                  pT_ps[:])
            elif TAIL_PE:
                psrc_sb = cp.tile([SP, KB, 128], F32)
                for a0, a1 in ((0, 4), (4, 8), (8, 10)):
                    if b1_zero:
                        nc.scalar.copy(psrc_sb[:, a0:a1, :],
                                       pooled_ps[0:SP, a0:a1, :])
                    else:
                        nc.vector.tensor_scalar_mul(
                            psrc_sb[:, a0:a1, :],
                            pooled_ps[0:SP, a0:a1, :], rz_sb[:])
                with tc.tile_pool(name="ptps", bufs=2, space="PSUM") as ptp:
                    for db in range(KB):
                        pT_ps = ptp.tile([128, SP], F32)
                        nc.tensor.transpose(pT_ps[:], psrc_sb[:, db, :],
                                            identS[:])
                        nc.scalar.copy(pT_sb[:, db, 0:SP], pT_ps[:])
            else:
                pTf_sb = cp.tile([128, KB, SP], F32)
                pn_sb = cp.tile([SP, KB, 128], F32)
                nc.vector.tensor_scalar_mul(
                    pn_sb[:, :, :], pooled_ps[0:SP, 0:KB, :], rz_sb[:])
                for t_ in range(KB):
                    for a in range(4):
                        nc.vector.transpose(
                            pTf_sb[32 * a:32 * a + 32, t_, :],
                            pn_sb[0:SP, t_, 32 * a:32 * a + 32])
                nc.scalar.copy(pT_sb[:, :, :], pTf_sb[:, :, :])

        if SPLIT:
            # boundary-segment partials for the host combine (raw bf16
            # pooled^T and clamped z); overlaps the head matmuls
            nc.scalar.dma_start(
                out=t["pt"][:],
                in_=pT_sb[:, :, :].rearrange("p kb s -> p (kb s)"))
        with tc.tile_pool(name="headps", bufs=2, space="PSUM") as headp:
            HH = DH // 2
            hnh_ps = [headp.tile([SP, HH], F32, name="hna"),
                      headp.tile([SP, HH], F32, name="hnb")]
            hn_sb = cp.tile([SP, DH], BF16)
            warmh_ps = headp.tile([SP, 512], F32, name="warmh")
            for hf in range(2):
                for db in range(KB):
                    if WARMKEEP and db % 5 == 0:
                        # transposes/copies don't count as PE-busy for HAM;
                        # keep K=8/8 through the head phase
                        nc.tensor.matmul(warmh_ps[:], zo_sb[:, 0:SP],
                                         zo_sb[:, 0:512], start=True,
                                         stop=True, skip_group_check=True)
                    nc.tensor.matmul(hnh_ps[hf][:],
                                     pT_sb[:, db, 0:SP],
                                     w1t_sb[:, db, hf * HH:(hf + 1) * HH],
                                     start=(db == 0), stop=(db == KB - 1))
                if not b1_zero:
                    nc.vector.tensor_add(hnh_ps[hf][:], hnh_ps[hf][:],
                                         b1rep_sb[:, hf * HH:(hf + 1) * HH])
                # relu of half A overlaps half B's matmuls (distinct banks)
                nc.scalar.activation(hn_sb[:, hf * HH:(hf + 1) * HH],
                                     hnh_ps[hf][:], AF.Relu)
            prodh_sb = cp.tile([SP, DH], BF16)
            oraw_sb = cp.tile([SP, 1], F32)
            if USE_STT and b1_zero:
                # fold the deferred 1/z scale into the fused dot:
                # accum = sum((hn * rz) * w2) = out
                oval_sb = cp.tile([SP, 1], F32)
                nc.vector.scalar_tensor_tensor(
                    out=prodh_sb[:], in0=hn_sb[:], scalar=rz_sb[:],
                    in1=w2b_sb[:], op0=ALU.mult, op1=ALU.mult,
                    accum_out=oval_sb[:])
            elif USE_STT:
                nc.vector.scalar_tensor_tensor(
                    out=prodh_sb[:], in0=hn_sb[:], scalar=0.0,
                    in1=w2b_sb[:], op0=ALU.bypass, op1=ALU.mult,
                    accum_out=oraw_sb[:])
                oval_sb = oraw_sb
            else:
                nc.vector.tensor_mul(prodh_sb[:], hn_sb[:], w2b_sb[:])
                nc.vector.tensor_reduce(oraw_sb[:], prodh_sb[:],
                                        axis=mybir.AxisListType.X,
                                        op=ALU.add)
                if b1_zero:
                    oval_sb = cp.tile([SP, 1], F32)
                    nc.vector.tensor_mul(oval_sb[:], oraw_sb[:], rz_sb[:])
                else:
                    oval_sb = oraw_sb
            # out straight from partition layout [S, 1]
            nc.sync.dma_start(out=t["out"][0:S, 0:1], in_=oval_sb[0:S, 0:1])


def _build_program(T_pad: int, S: int, b1_zero: bool, bw_zero: bool):
    key = (T_pad, S, b1_zero, bw_zero, USE_FP8, TAIL_PE, USE_STT, NEW_TAIL,
           WARM_REPS, SPLIT)
    if key in _PROGRAM_CACHE:
        return _PROGRAM_CACHE[key]
    NB = T_pad // 128
    mmdt = FP8 if USE_FP8 else BF16
    nc = bacc.Bacc("TRN2", target_bir_lowering=False, debug=False,
                   num_devices=N_CORES)
    t = {
        "xt": nc.dram_tensor("xt", [128, NB * KB * 128], mmdt,
                             kind="ExternalInput").ap(),
        "xn": nc.dram_tensor("xn", [T_pad, D], BF16, kind="ExternalInput").ap(),
        "wwt": nc.dram_tensor("wwt", [D, D], mmdt, kind="ExternalInput").ap(),
        "m1h": nc.dram_tensor("m1h", [128, NB * S], BF16,
                              kind="ExternalInput").ap(),
        "vr": nc.dram_tensor("vr", [128, D], BF16, kind="ExternalInput").ap(),
        "bwr": nc.dram_tensor("bwr", [128, D], F32,
                              kind="ExternalInput").ap(),
        "w1t": nc.dram_tensor("w1t", [D, DH], BF16, kind="ExternalInput").ap(),
        "b1r": nc.dram_tensor("b1r", [SP, DH], F32, kind="ExternalInput").ap(),
        "w2r": nc.dram_tensor("w2r", [SP, DH], BF16,
                              kind="ExternalInput").ap(),
        "out": nc.dram_tensor("out", [SP, 1], F32, kind="ExternalOutput").ap(),
    }
    if SPLIT:
        t["pt"] = nc.dram_tensor("pt", [128, KB * SP], BF16,
                                 kind="ExternalOutput").ap()
        t["zs"] = nc.dram_tensor("zs", [SP, 1], F32,
                                 kind="ExternalOutput").ap()
    with tile.TileContext(nc) as tc:
        _emit(tc, t, T_pad, S, b1_zero, bw_zero)
    nc.compile()
    _PROGRAM_CACHE[key] = nc
    return nc


def kernel(features, Ww, bw, v, W1, b1, W2, b2, segment_ids):
    global LAST_RESULTS
    features = np.ascontiguousarray(np.asarray(features, dtype=np.float32))
    Ww = np.asarray(Ww, dtype=np.float32)
    bw = np.asarray(bw, dtype=np.float32)
    v = np.asarray(v, dtype=np.float32)
    W1 = np.asarray(W1, dtype=np.float32)
    b1 = np.asarray(b1, dtype=np.float32)
    W2 = np.asarray(W2, dtype=np.float32)
    b2 = np.asarray(b2, dtype=np.float32)
    segment_ids = np.asarray(segment_ids)

    seg64 = segment_ids.astype(np.int64)
    total = seg64.shape[0]
    lengths = np.bincount(seg64, minlength=N_SEG)[:N_SEG]
    seg_prefix = np.concatenate([[0], np.cumsum(lengths)])

    if SPLIT and total % N_CORES == 0:
        # exact contiguous token ranges; segments may split at boundaries
        tok_cuts = [total // N_CORES * c for c in range(N_CORES + 1)]
        seg_lo = [int(seg64[tok_cuts[c]]) for c in range(N_CORES)]
        seg_hi = [int(seg64[tok_cuts[c + 1] - 1]) for c in range(N_CORES)]
        S = max(seg_hi[c] - seg_lo[c] + 1 for c in range(N_CORES))
    else:
        cuts = _partition_segments(lengths)
        tok_cuts = [int(seg_prefix[c]) for c in cuts]
        seg_lo = [cuts[c] for c in range(N_CORES)]
        seg_hi = [cuts[c + 1] - 1 for c in range(N_CORES)]
        S = max(cuts[c + 1] - cuts[c] for c in range(N_CORES))
    assert S <= SP, f"segments per core {S} exceeds {SP}"
    T_max = max(tok_cuts[c + 1] - tok_cuts[c] for c in range(N_CORES))
    T_pad = max(512, ((T_max + 127) // 128) * 128)
    NB = T_pad // 128

    b1_zero = bool(np.all(b1 == 0))
    bw_zero = bool(np.all(bw == 0))

    mmnp = NPF8 if USE_FP8 else NPBF
    wsc = WSCALE if USE_FP8 else 1.0
    wwt = np.ascontiguousarray((Ww.T * wsc)).astype(mmnp)      # [k, m]
    vr = np.tile(v.reshape(1, D), (128, 1)).astype(NPBF)
    bwr = np.tile((bw.reshape(1, D) * wsc), (128, 1)).astype(np.float32)
    w1t = np.ascontiguousarray(W1.T).astype(NPBF)              # [k, h]
    b1r = np.tile(b1.reshape(1, DH), (SP, 1)).astype(np.float32)
    w2r = np.tile(W2[0:1, :], (SP, 1)).astype(NPBF)

    in_maps = []
    for c in range(N_CORES):
        t0, t1 = tok_cuts[c], tok_cuts[c + 1]
        Tc = t1 - t0
        xn = np.zeros((T_pad, D), dtype=np.float32)
        xn[:Tc] = features[t0:t1]
        # chunk-major X.T: xt[p, ((c*KB + kb)*128 + n)] = xn[c*128+n,
        # kb*128+p]
        xt = np.ascontiguousarray(
            xn.reshape(NB, 128, KB, 128).transpose(3, 0, 2, 1)
        ).astype(mmnp).reshape(128, NB * KB * 128)
        oh = np.zeros((T_pad, S), dtype=np.float32)
        if Tc > 0:
            loc = seg64[t0:t1] - seg_lo[c]
            ok = (loc >= 0) & (loc < S)
            oh[np.arange(Tc)[ok], loc[ok]] = 1.0
        m1h = np.ascontiguousarray(
            oh.reshape(NB, 128, S).transpose(1, 0, 2).reshape(128, NB * S)
        ).astype(NPBF)
        in_maps.append({
            "xt": xt, "xn": xn.astype(NPBF), "m1h": m1h,
            "wwt": wwt, "vr": vr, "bwr": bwr,
            "w1t": w1t, "b1r": b1r, "w2r": w2r,
        })

    nc = _build_program(T_pad, S, b1_zero, bw_zero)
    trace = bool(int(os.environ.get("KERNEL_TRACE", "0")))
    res = run_bass_kernel_spmd(nc, in_maps, core_ids=list(range(N_CORES)),
                               trace=trace)
    LAST_RESULTS = res

    out = np.zeros((N_SEG, 1), dtype=np.float32)
    for c in range(N_CORES):
        n = seg_hi[c] - seg_lo[c] + 1
        out[seg_lo[c]:seg_hi[c] + 1, 0] = res.results[c]["out"][:n, 0]

    if SPLIT and total % N_CORES == 0:
        # segments split across a core boundary: sum the pooled/z partials
        # and redo those head rows on the host
        boundary = sorted({seg_lo[c] for c in range(1, N_CORES)
                           if seg_lo[c] == seg_hi[c - 1]})
        for j in boundary:
            pooled = np.zeros(D, dtype=np.float32)
            z = 0.0
            for c in range(N_CORES):
                if seg_lo[c] <= j <= seg_hi[c]:
                    loc = j - seg_lo[c]
                    pt = np.asarray(res.results[c]["pt"],
                                    dtype=np.float32)  # [128, KB*SP]
                    pooled += pt.reshape(128, KB, SP)[:, :, loc] \
                        .T.reshape(D)
                    z += float(res.results[c]["zs"][loc, 0])
            z = max(z, 1e-30)
            h = np.maximum(pooled / z @ W1.T + b1, 0.0)
            out[j, 0] = float(h @ W2[0])
    out[:, 0] += b2[0]

    # empty segments: pooled = 0 -> out = relu(b1) @ W2.T + b2 (host patch;
    # device row may be garbage from 0 * (1/0))
    empty = lengths == 0
    if empty.any():
        out[empty, 0] = float(np.maximum(b1, 0.0) @ W2[0] + b2[0])
    return out
